# revision 6
# baseline (speedup 1.0000x reference)
"""Trainium2 Bass kernel for the NMS-BP decoder — dual-stream pipelined edition.

Self-contained: takes the FULL inputs of reference.setup_inputs(), shards the
batch across 8 NeuronCores (pure data parallelism), runs a Bass/Tile NEFF per
core, and reassembles the full [6, 64, 1024] output.

Per core the 8-row batch is split into TWO independent 4-row streams that the
Tile list-scheduler interleaves: while stream A runs its DVE chain (sort + NN
masks), stream B occupies the PE (gather/colsum one-hot routing matmuls), so
the serial decode dependency of one stream hides under the other stream's
compute. All arithmetic stays bit-exact fp32 (the decoder's sign() chain is
chaotic: even 1e-5 perturbations flip signs and blow past the error gate):

  * routing runs on the TENSOR engine as one-hot fp8 matmuls with the moving
    data packed as a bf16 TRIPLE (h, m, l; h+m+l == x exactly in fp32);
  * the gather routes the column-sum cs only: temp = cs + c1 never
    materializes, because g_c1 = (sp1*soft)[cols] is precomputed EXACTLY on
    the host and DMA'd in.  Iteration 1 (cv = 0, cs = 0) therefore needs no
    gather at all: vc(1) = g_c1, so DVE work starts as soon as g_c1 lands —
    and the big one-hot weight tensors stream in chunks during iteration 1
    instead of gating the kernel start.

Checks are reassigned to (mhi, mlow) positions sorted by mean column index,
which concentrates each q-plane's columns into few 128-column chunks: only
~104 of 192 (q, k) incidence tiles are nonzero and empty tiles are skipped.

The 12-comparator 6-lane sorting network runs as 5 fused layers; physical
j-planes hold logical edge lanes in LOGMAP order so layer 1 is a contiguous
half-vs-half min/max. abs/sign run as single Activation-engine ops, the sign
products and sign*psign on GpSimd, and the w_k scalings as scaled Activation
copies, so DVE keeps only the critical chain.

Layouts (per stream, SBL = 4 batch rows):
  check/slot domain: [128 p = mlow, 24 q = jp*4 + mhi, SBL b]; slot s = q*128+p,
  col(s) = row_cols[assign[(q%4)*128 + p], LOGMAP[q//4]].
  column domain:     [128 p = nlow, 8 k, SBL b]; column n = k*128 + p.
"""

import numpy as np

B, N, M, DC, NUM_ITERS = 64, 1024, 512, 6, 5
NCORES = 8
BL = B // NCORES          # 8 batch rows per core
NS = 2                    # streams per core
SBL = BL // NS            # 4 batch rows per stream
NSLOT = M * DC            # 3072
LOGMAP = [0, 1, 2, 5, 3, 4]   # physical j-plane -> logical (sorted-col) lane
WG_CHUNKS = 8
WC_CHUNKS = 4

_CACHE = {}


def _layout(row_cols):
    """Check assignment (sorted by mean col) + per-slot columns + tile lists."""
    assign = np.argsort(row_cols.mean(axis=1), kind="stable")  # position -> check
    cols = np.empty(NSLOT, np.int64)
    for q in range(24):
        jp, mhi = q // 4, q % 4
        j = LOGMAP[jp]
        for p in range(128):
            cols[q * 128 + p] = row_cols[assign[mhi * 128 + p], j]
    present = [sorted({int(c) // 128 for c in cols[q * 128:(q + 1) * 128]})
               for q in range(24)]
    gt = [(q, k) for q in range(24) for k in present[q]]           # gather tiles
    ct = [(k, q) for k in range(8) for q in range(24) if k in present[q]]
    return assign, cols, present, gt, ct


def _weights(cols, gt, ct):
    wg = np.zeros((128, len(gt), 128), np.float32)
    for t, (q, k) in enumerate(gt):
        for po in range(128):
            c = cols[q * 128 + po]
            if c // 128 == k:
                wg[c % 128, t, po] = 1.0
    wc = np.zeros((128, len(ct), 128), np.float32)
    for t, (k, q) in enumerate(ct):
        for ps in range(128):
            c = cols[q * 128 + ps]
            if c // 128 == k:
                wc[ps, t, c % 128] = 1.0
    return wg, wc


def _chunk_bounds(n, nchunks):
    base, rem = divmod(n, nchunks)
    bounds = [0]
    for i in range(nchunks):
        bounds.append(bounds[-1] + base + (1 if i < rem else 0))
    return bounds


def _build(cols, w, sp1, sp2, gt, ct):
    import concourse.bass as bass
    import concourse.bacc as bacc
    import concourse.tile as tile
    import concourse.mybir as mybir

    dt = mybir.dt
    Alu = mybir.AluOpType
    ActF = mybir.ActivationFunctionType
    f32 = dt.float32
    bf16 = dt.bfloat16
    f8 = dt.float8e4

    nc = bacc.Bacc("TRN2", target_bir_lowering=False, debug=False)

    NGT, NCT = len(gt), len(ct)
    soft_t = nc.dram_tensor("soft_t", [N, BL], f32, kind="ExternalInput")
    gc1_d = nc.dram_tensor("gc1", [128, 24 * BL], f32, kind="ExternalInput")
    wg_d = nc.dram_tensor("wg", [128, NGT * 128], f8, kind="ExternalInput")
    wc_d = nc.dram_tensor("wc", [128, NCT * 128], f8, kind="ExternalInput")
    out = nc.dram_tensor("out", [NUM_ITERS + 1, N, BL], f32, kind="ExternalOutput")

    w = [float(x) for x in w]
    sp1 = float(sp1)
    sp2 = float(sp2)

    gb = _chunk_bounds(NGT, WG_CHUNKS)
    cb = _chunk_bounds(NCT, WC_CHUNKS)

    def g_chunk(t):
        for i in range(WG_CHUNKS):
            if gb[i] <= t < gb[i + 1]:
                return i, t - gb[i]
        raise AssertionError

    def c_chunk(t):
        for i in range(WC_CHUNKS):
            if cb[i] <= t < cb[i + 1]:
                return i, t - cb[i]
        raise AssertionError

    gt_pos = {qk: t for t, qk in enumerate(gt)}
    ct_pos = {kq: t for t, kq in enumerate(ct)}
    pres_q = {}
    for (q, k) in gt:
        pres_q.setdefault(q, []).append(k)
    pres_k = {}
    for (k, q) in ct:
        pres_k.setdefault(k, []).append(q)

    with tile.TileContext(nc) as tc:
        with (
            tc.tile_pool(name="const", bufs=1) as pc,
            tc.tile_pool(name="work", bufs=2) as pw,
            tc.tile_pool(name="srt", bufs=12) as psrt,
            tc.tile_pool(name="small", bufs=24) as psm,
            tc.tile_pool(name="ppg", bufs=2, space="PSUM") as ppg,
            tc.tile_pool(name="ppc", bufs=2, space="PSUM") as ppc,
        ):
            # ---- input DMAs: small tensors first, then weights in chunks ----
            sT = pc.tile([128, 8, BL], f32)
            nc.sync.dma_start(sT[:, :, :], soft_t.rearrange("(nh p) b -> p nh b", p=128))
            gc1 = pc.tile([128, 24, BL], f32)
            nc.sync.dma_start(gc1[:, :, :].rearrange("p q b -> p (q b)"), gc1_d[:, :])
            nc.sync.dma_start(out[0][:, :], soft_t[:, :])

            # wc before wg: colsum (it=1) runs before gather (it=2)
            wc_sb = []
            for i in range(WC_CHUNKS):
                sz = cb[i + 1] - cb[i]
                t_ = pc.tile([128, sz, 128], f8, tag=f"wc{i}", name=f"wc{i}")
                nc.sync.dma_start(
                    t_[:, :, :].rearrange("p a c -> p (a c)"),
                    wc_d[:, cb[i] * 128:cb[i + 1] * 128])
                wc_sb.append(t_)
            wg_sb = []
            for i in range(WG_CHUNKS):
                sz = gb[i + 1] - gb[i]
                t_ = pc.tile([128, sz, 128], f8, tag=f"wg{i}", name=f"wg{i}")
                nc.sync.dma_start(
                    t_[:, :, :].rearrange("p a c -> p (a c)"),
                    wg_d[:, gb[i] * 128:gb[i + 1] * 128])
                wg_sb.append(t_)

            c2 = pc.tile([128, 8, BL], f32)
            nc.any.tensor_scalar(c2[:, :, :], sT[:, :, :], sp2, None, Alu.mult)

            def split_tri(src_f32, tri, nmid, tag):
                """tri[:, :, 0..2, :] = bf16 triple of src (h, m, l)."""
                nc.vector.tensor_copy(tri[:, :, 0, :], src_f32)
                r = pw.tile([128, nmid, SBL], f32, tag=f"r{tag}", name="r")
                nc.vector.tensor_tensor(r[:, :, :], src_f32, tri[:, :, 0, :], Alu.subtract)
                nc.vector.tensor_copy(tri[:, :, 1, :], r[:, :, :])
                nc.vector.tensor_tensor(tri[:, :, 2, :], r[:, :, :], tri[:, :, 1, :], Alu.subtract)

            def pl(t, i, n=1):
                """n plane-groups of 4 starting at plane i."""
                return t[:, 4 * i:4 * (i + n), :]

            def g3(t, gidx):
                """planes (gidx, gidx+3) as [128, 2, 4, SBL] (stride-3 pair)."""
                return t[:, :, :].rearrange("p (two g m) b -> p two g m b", two=2, g=3)[:, :, gidx, :, :]

            def w2(t, i):
                """planes (i, i+2) as [128, 2, 4, SBL] (stride-2 pair window)."""
                return t[:, 4 * i:4 * i + 16, :].rearrange(
                    "p (two g m) b -> p two g m b", two=2, g=2)[:, :, 0, :, :]

            # per-stream persistent state across iterations
            cs_tri = [None] * NS   # bf16 triple of cs (gather moving data)
            cv = [None] * NS       # previous iteration's cv
            qm = [None] * NS       # g_c1 - cv (precomputed off critical path)

            for it in range(1, NUM_ITERS + 1):
                for s in range(NS):
                    bsl = slice(s * SBL, (s + 1) * SBL)
                    gc1_s = gc1[:, :, bsl]
                    c2_s = c2[:, :, bsl]

                    # ---- gather phase (it >= 2): route cs through one-hot PE ----
                    if it == 1:
                        vc = gc1_s  # vc(1) = (sp1*soft)[cols], exact
                    else:
                        vc_ps = ppg.tile([128, 24, 3, SBL], f32, tag=f"vps{s}", name=f"vps{s}")
                        for q in range(24):
                            o = vc_ps[:, q, :, :].rearrange("p t b -> p (t b)")
                            ks = pres_q[q]
                            for i, k in enumerate(ks):
                                ci, off = g_chunk(gt_pos[(q, k)])
                                nc.tensor.matmul(
                                    o, wg_sb[ci][:, off, :],
                                    cs_tri[s][:, k, :, :].rearrange("p t b -> p (t b)"),
                                    start=(i == 0), stop=(i == len(ks) - 1))
                        g1 = pw.tile([128, 24, SBL], f32, tag=f"g1{s}", name="g1")
                        nc.vector.tensor_tensor(g1[:, :, :], vc_ps[:, :, 0, :], qm[s][:, :, :], Alu.add)
                        g2 = pw.tile([128, 24, SBL], f32, tag=f"g2{s}", name="g2")
                        nc.vector.tensor_tensor(g2[:, :, :], g1[:, :, :], vc_ps[:, :, 1, :], Alu.add)
                        vcf = pw.tile([128, 24, SBL], f32, tag=f"vc{s}", name="vc")
                        nc.vector.tensor_tensor(vcf[:, :, :], g2[:, :, :], vc_ps[:, :, 2, :], Alu.add)
                        vc = vcf

                    # ---- vector phase ----
                    a = pw.tile([128, 24, SBL], f32, tag=f"a{s}")
                    nc.scalar.activation(a[:, :, :], vc[:, :, :], ActF.Abs)
                    sg = pw.tile([128, 24, SBL], f32, tag=f"sg{s}")
                    nc.scalar.activation(sg[:, :, :], vc[:, :, :], ActF.Sign)

                    # psign on gpsimd (parallel with DVE sort)
                    p1 = psm.tile([128, 12, SBL], f32, tag=f"p1{s}")
                    nc.gpsimd.tensor_tensor(p1[:, :, :], sg[:, 0:12, :], sg[:, 12:24, :], Alu.mult)
                    p2 = psm.tile([128, 4, SBL], f32, tag=f"p2{s}")
                    nc.gpsimd.tensor_tensor(p2[:, :, :], p1[:, 0:4, :], p1[:, 4:8, :], Alu.mult)
                    ps = psm.tile([128, 4, SBL], f32, tag=f"ps{s}")
                    nc.gpsimd.tensor_tensor(ps[:, :, :], p2[:, :, :], p1[:, 8:12, :], Alu.mult)

                    # ---- fused 5-layer sort (physical planes hold LOGMAP lanes) ----
                    T1 = psrt.tile([128, 24, SBL], f32, tag=f"T1{s}", name="T1")
                    nc.vector.tensor_tensor(pl(T1, 0, 3), pl(a, 0, 3), pl(a, 3, 3), Alu.min)
                    nc.vector.tensor_tensor(pl(T1, 3, 3), pl(a, 0, 3), pl(a, 3, 3), Alu.max)
                    # T1 planes = [pos0, pos1, pos2, pos5, pos3, pos4]
                    T2 = psrt.tile([128, 24, SBL], f32, tag=f"T2{s}", name="T2")
                    nc.vector.tensor_tensor(w2(T2, 1), g3(T1, 1), g3(T1, 2), Alu.min)
                    nc.vector.tensor_tensor(w2(T2, 2), g3(T1, 1), g3(T1, 2), Alu.max)
                    # T2 planes (1..4) = [pos1, pos2, pos3, pos4]; pos0 @ T1[0], pos5 @ T1[3]
                    T3 = psrt.tile([128, 24, SBL], f32, tag=f"T3{s}", name="T3")
                    nc.vector.tensor_tensor(pl(T3, 0), pl(T1, 0), pl(T2, 3), Alu.min)
                    nc.vector.tensor_tensor(pl(T3, 4), pl(T1, 0), pl(T2, 3), Alu.max)
                    nc.vector.tensor_tensor(pl(T3, 1), pl(T2, 2), pl(T1, 3), Alu.min)
                    nc.vector.tensor_tensor(pl(T3, 5), pl(T2, 2), pl(T1, 3), Alu.max)
                    nc.scalar.activation(pl(T3, 2), pl(T2, 4), ActF.Copy, scale=1.0)
                    nc.scalar.activation(pl(T3, 3), pl(T2, 1), ActF.Copy, scale=1.0)
                    # T3 planes = [pos0, pos2, pos4, pos1, pos3, pos5]
                    T4 = psrt.tile([128, 24, SBL], f32, tag=f"T4{s}", name="T4")
                    nc.vector.tensor_tensor(pl(T4, 0, 3), pl(T3, 0, 3), pl(T3, 3, 3), Alu.min)
                    nc.vector.tensor_tensor(pl(T4, 3, 3), pl(T3, 0, 3), pl(T3, 3, 3), Alu.max)
                    S13 = psrt.tile([128, 8, SBL], f32, tag=f"S13{s}", name="S13")
                    nc.vector.tensor_tensor(S13[:, :, :], pl(T4, 3, 2), pl(T4, 1, 2), Alu.min)
                    S24 = psrt.tile([128, 8, SBL], f32, tag=f"S24{s}", name="S24")
                    nc.vector.tensor_tensor(S24[:, :, :], pl(T4, 3, 2), pl(T4, 1, 2), Alu.max)
                    lanes = [pl(T4, 0), S13[:, 0:4, :], S24[:, 0:4, :],
                             S13[:, 4:8, :], S24[:, 4:8, :], pl(T4, 5)]

                    # u_k = w_k s_k (Act, scaled copies); base tree on any
                    u = []
                    for kk in range(5):
                        uk = psm.tile([128, 4, SBL], f32, tag=f"u{kk}{s}", name=f"uk{kk}")
                        nc.scalar.activation(uk[:, :, :], lanes[kk], ActF.Copy, scale=w[kk])
                        u.append(uk)
                    b01 = psm.tile([128, 4, SBL], f32, tag=f"b01{s}")
                    nc.any.tensor_tensor(b01[:, :, :], u[0][:, :, :], u[1][:, :, :], Alu.add)
                    b23 = psm.tile([128, 4, SBL], f32, tag=f"b23{s}")
                    nc.any.tensor_tensor(b23[:, :, :], u[2][:, :, :], u[3][:, :, :], Alu.add)
                    b03 = psm.tile([128, 4, SBL], f32, tag=f"b03{s}")
                    nc.any.tensor_tensor(b03[:, :, :], b01[:, :, :], b23[:, :, :], Alu.add)
                    base = psm.tile([128, 4, SBL], f32, tag=f"base{s}")
                    nc.any.tensor_tensor(base[:, :, :], b03[:, :, :], u[4][:, :, :], Alu.add)

                    # e_k = w_k (s_{k+1} - s_k): diff on DVE, scale on Act
                    e = []
                    for kk in range(5):
                        dk = psm.tile([128, 4, SBL], f32, tag=f"d{kk}{s}", name=f"dk{kk}")
                        nc.vector.tensor_tensor(dk[:, :, :], lanes[kk + 1], lanes[kk], Alu.subtract)
                        ek = psm.tile([128, 4, SBL], f32, tag=f"e{kk}{s}", name=f"ek{kk}")
                        nc.scalar.activation(ek[:, :, :], dk[:, :, :], ActF.Copy, scale=w[kk])
                        e.append(ek)

                    a4 = a[:, :, :].rearrange("p (j m) b -> p j m b", j=DC)
                    bshape = [128, DC, 4, SBL]
                    terms = []
                    for kk in range(5):
                        cmp = pw.tile([128, 24, SBL], f32, tag=f"cmp{kk}{s}", name=f"cmp{kk}")
                        cmp4 = cmp[:, :, :].rearrange("p (j m) b -> p j m b", j=DC)
                        sk_b = lanes[kk].unsqueeze(1).broadcast_to(bshape)
                        nc.vector.tensor_tensor(cmp4, sk_b, a4, Alu.is_ge)
                        ek_b = e[kk][:, :, :].unsqueeze(1).broadcast_to(bshape)
                        nc.vector.tensor_tensor(cmp4, cmp4, ek_b, Alu.mult)
                        terms.append(cmp)
                    t01 = pw.tile([128, 24, SBL], f32, tag=f"t01{s}")
                    nc.vector.tensor_tensor(t01[:, :, :], terms[0][:, :, :], terms[1][:, :, :], Alu.add)
                    t23 = pw.tile([128, 24, SBL], f32, tag=f"t23{s}")
                    nc.vector.tensor_tensor(t23[:, :, :], terms[2][:, :, :], terms[3][:, :, :], Alu.add)
                    t4b = pw.tile([128, 24, SBL], f32, tag=f"t4b{s}")
                    t4b4 = t4b[:, :, :].rearrange("p (j m) b -> p j m b", j=DC)
                    nc.vector.tensor_tensor(
                        t4b4, terms[4][:, :, :].rearrange("p (j m) b -> p j m b", j=DC),
                        base[:, :, :].unsqueeze(1).broadcast_to(bshape), Alu.add)
                    t0123 = pw.tile([128, 24, SBL], f32, tag=f"t0123{s}")
                    nc.vector.tensor_tensor(t0123[:, :, :], t01[:, :, :], t23[:, :, :], Alu.add)
                    acc = pw.tile([128, 24, SBL], f32, tag=f"acc{s}")
                    nc.vector.tensor_tensor(acc[:, :, :], t0123[:, :, :], t4b[:, :, :], Alu.add)

                    # sg_loo = sg * psign on gpsimd (off the DVE chain)
                    sg_loo = pw.tile([128, 24, SBL], f32, tag=f"sgloo{s}")
                    sgl4 = sg_loo[:, :, :].rearrange("p (j m) b -> p j m b", j=DC)
                    sg4 = sg[:, :, :].rearrange("p (j m) b -> p j m b", j=DC)
                    ps_b = ps[:, :, :].unsqueeze(1).broadcast_to(bshape)
                    nc.gpsimd.tensor_tensor(sgl4, sg4, ps_b, Alu.mult)
                    cvn = pw.tile([128, 24, SBL], f32, tag=f"cv{s}", name="cv")
                    nc.vector.tensor_tensor(cvn[:, :, :], acc[:, :, :], sg_loo[:, :, :], Alu.mult)
                    cv[s] = cvn

                    # qm = g_c1 - cv for the NEXT iteration's reconstruct
                    # (ready now; runs during the colsum PE phase)
                    if it < NUM_ITERS:
                        qmn = pw.tile([128, 24, SBL], f32, tag=f"qm{s}", name="qm")
                        nc.vector.tensor_tensor(qmn[:, :, :], gc1_s, cvn[:, :, :], Alu.subtract)
                        qm[s] = qmn

                    # ---- split + colsum ----
                    cv_tri = pw.tile([128, 24, 3, SBL], bf16, tag=f"cvtri{s}", name="cv_tri")
                    split_tri(cvn[:, :, :], cv_tri, 24, f"cv{s}")
                    cs_ps = ppc.tile([128, 8, 3, SBL], f32, tag=f"cps{s}", name=f"cps{s}")
                    for k in range(8):
                        o = cs_ps[:, k, :, :].rearrange("p t b -> p (t b)")
                        qs = pres_k[k]
                        for i, q in enumerate(qs):
                            ci, off = c_chunk(ct_pos[(k, q)])
                            nc.tensor.matmul(
                                o, wc_sb[ci][:, off, :],
                                cv_tri[:, q, :, :].rearrange("p t b -> p (t b)"),
                                start=(i == 0), stop=(i == len(qs) - 1))

                    csh = pw.tile([128, 8, SBL], f32, tag=f"csh{s}", name="csh")
                    nc.vector.tensor_copy(csh[:, :, :], cs_ps[:, :, 0, :])
                    csm = pw.tile([128, 8, SBL], f32, tag=f"csm{s}", name="csm")
                    nc.vector.tensor_tensor(csm[:, :, :], csh[:, :, :], cs_ps[:, :, 1, :], Alu.add)
                    cs = pw.tile([128, 8, SBL], f32, tag=f"cs{s}", name="cs")
                    nc.vector.tensor_tensor(cs[:, :, :], csm[:, :, :], cs_ps[:, :, 2, :], Alu.add)

                    so = pw.tile([128, 8, SBL], f32, tag=f"so{s}", name="so")
                    nc.any.tensor_tensor(so[:, :, :], cs[:, :, :], c2_s, Alu.add)
                    nc.sync.dma_start(
                        out[it].rearrange("(nh p) b -> p nh b", p=128)[:, :, bsl],
                        so[:, :, :])

                    if it < NUM_ITERS:
                        # gather(it+1) routes cs only; the c1 part arrives via
                        # the host-precomputed g_c1 (qm = g_c1 - cv above)
                        tri = pw.tile([128, 8, 3, SBL], bf16, tag=f"cstri{s}", name="cstri")
                        split_tri(cs[:, :, :], tri, 8, f"cs{s}")
                        cs_tri[s] = tri

    nc.compile()
    return nc


def _get_nc(row_cols, W1, W2, bit_w1, bit_w2):
    row_cols = np.asarray(row_cols)
    w = (np.asarray(W1, np.float32) @ np.asarray(W2, np.float32))[:, 0]
    sp1 = float(np.log1p(np.exp(np.asarray(bit_w1, np.float32)))[0])
    sp2 = float(np.log1p(np.exp(np.asarray(bit_w2, np.float32)))[0])
    key = (row_cols.tobytes(), w.tobytes(), sp1, sp2)
    if key not in _CACHE:
        import ml_dtypes
        assign, cols, present, gt, ct = _layout(row_cols)
        wg, wc = _weights(cols, gt, ct)
        f8 = ml_dtypes.float8_e4m3fn
        _CACHE[key] = (_build(cols, w, sp1, sp2, gt, ct),
                       np.ascontiguousarray(wg.reshape(128, -1).astype(f8)),
                       np.ascontiguousarray(wc.reshape(128, -1).astype(f8)),
                       cols, sp1)
    return _CACHE[key]


def _in_maps(inputs):
    soft = np.asarray(inputs["soft_input"], np.float32)
    nc, wg, wc, cols, sp1 = _get_nc(inputs["row_cols"], inputs["W1"], inputs["W2"],
                                    inputs["bit_w1"], inputs["bit_w2"])
    in_maps = []
    for c in range(NCORES):
        shard = soft[c * BL:(c + 1) * BL, :]  # [8, 1024]
        c1 = (shard * np.float32(sp1)).astype(np.float32)   # [8, 1024] exact fp32
        # g_c1[p, q, b] = c1[b, cols[q*128 + p]]
        g = c1[:, cols.reshape(24, 128)]          # [8, 24, 128]
        g = np.ascontiguousarray(g.transpose(2, 1, 0).reshape(128, 24 * BL))
        in_maps.append({
            "soft_t": np.ascontiguousarray(shard.T),  # [1024, 8]
            "gc1": g.astype(np.float32),
            "wg": wg,
            "wc": wc,
        })
    return nc, in_maps


def kernel(**inputs):
    from concourse.bass_utils import run_bass_kernel_spmd

    nc, in_maps = _in_maps(inputs)
    res = run_bass_kernel_spmd(nc, in_maps, core_ids=list(range(NCORES)))

    full = np.empty((NUM_ITERS + 1, B, N), np.float32)
    for c in range(NCORES):
        o = res.results[c]["out"]  # [6, 1024, 8]
        full[:, c * BL:(c + 1) * BL, :] = o.transpose(0, 2, 1)
    return full
